# revision 33
# baseline (speedup 1.0000x reference)
"""Multi-head attention layer (B=4, S=2048, H=8, D=128) on 8 trn2 NeuronCores.

Sharding: core c handles batch b = c//2 and query half c%2 (1024 query rows).
Each core computes K/V over all 2048 keys of its batch, its 1024x2048 slice of
the attention matrix for all 8 heads, and its slice of the final
residual+LayerNorm output.  Outputs are assembled on the host by pure
concatenation - no cross-core reduction.

Pipeline: heads stream one at a time; head h+1's Q/K/V projections overlap
head h's softmax/PV work (all PSUM pools are shared, so nothing serializes on
allocation).  The random 0/1 int32 mask is DMA-cast to fp16 once up front by
the gpsimd DGE.  Score chain (projections + Q@K) runs in float32r (full-rate
fp32 with ~1e-4 rounding); the value chain (exp output, attn weights, V, Wo)
runs fp16; the attention-probability output, residual and LayerNorm are fp32.
The big probability transpose for attn@V rides the DMA XBAR transpose engine
instead of the tensor engine.
"""

import math
from contextlib import ExitStack

import numpy as np

import concourse.bass as bass
import concourse.mybir as mybir
import concourse.tile as tile
from concourse import bacc
from concourse.bass_utils import run_bass_kernel_spmd
from concourse.masks import make_identity

B, S, D, H = 4, 2048, 128, 8
QS = S // 2            # 1024 query rows per core
NQT = QS // 128        # 8 query tiles per core
NKT = S // 128         # 16 key tiles
SCALE = 1.0 / math.sqrt(D)
LN_EPS = 1e-6

FP32 = mybir.dt.float32
FP16 = mybir.dt.float16
F32R = mybir.dt.float32r
I32 = mybir.dt.int32
AF = mybir.ActivationFunctionType
OP = mybir.AluOpType


def build_bass():
    nc = bacc.Bacc(trn_type="TRN2")

    enc_b = nc.dram_tensor("enc_b", [S, D], FP32, kind="ExternalInput")
    enc_q = nc.dram_tensor("enc_q", [QS, D], FP32, kind="ExternalInput")
    mask_p = nc.dram_tensor("mask_p", [QS, S], I32, kind="ExternalInput")
    wq_d = nc.dram_tensor("wq", [D, H * D], FP32, kind="ExternalInput")
    wk_d = nc.dram_tensor("wk", [D, H * D], FP32, kind="ExternalInput")
    wv_d = nc.dram_tensor("wv", [D, H * D], FP32, kind="ExternalInput")
    wo_d = nc.dram_tensor("wo", [H * D, D], FP32, kind="ExternalInput")
    lns_d = nc.dram_tensor("ln_scale", [D], FP32, kind="ExternalInput")
    lnb_d = nc.dram_tensor("ln_bias", [D], FP32, kind="ExternalInput")
    attn_o = nc.dram_tensor("attn_o", [H, QS, S], FP32, kind="ExternalOutput")
    out_o = nc.dram_tensor("out_o", [QS, D], FP32, kind="ExternalOutput")

    with TileKernel(nc) as tk:
        tk.run(enc_b, enc_q, mask_p, wq_d, wk_d, wv_d, wo_d, lns_d, lnb_d,
               attn_o, out_o)
    nc.finalize()
    return nc


class TileKernel:
    def __init__(self, nc):
        self.nc = nc
        self.ctx = ExitStack()
        self.tc = None

    def __enter__(self):
        self.tc = self.ctx.enter_context(tile.TileContext(self.nc))
        return self

    def __exit__(self, *exc):
        return self.ctx.__exit__(*exc)

    def run(self, enc_b, enc_q, mask_p, wq_d, wk_d, wv_d, wo_d, lns_d, lnb_d,
            attn_o, out_o):
        nc, tc, ctx = self.nc, self.tc, self.ctx

        consts = ctx.enter_context(tc.tile_pool(name="consts", bufs=1))

        # ---- persistent tiles ----
        wq_r = consts.tile([D, H * D], F32R)
        wk_r = consts.tile([D, H * D], F32R)
        wv_r = consts.tile([D, H * D], F32R)
        wo16 = consts.tile([D, H * D], FP16)
        lns_sb = consts.tile([128, D], FP32)
        lnb_sb = consts.tile([128, D], FP32)
        ident = consts.tile([128, 128], FP32)
        encq_sb = consts.tile([128, NQT * D], FP32)
        encT_sb = consts.tile([D, S], F32R)     # enc_b transposed [d, seq]
        encqT_sb = consts.tile([D, QS], F32R)   # enc_q transposed [d, q]
        acc_sb = consts.tile([128, NQT * D], FP32)  # sum_h attn@V@Wo, [q, d]
        mask_all = consts.tile([128, NQT * S], FP16)  # all 8 q-tiles of mask
        zero_bias = consts.tile([128, 1], FP32)
        eps_bias = consts.tile([128, 1], FP32)
        mu_all = consts.tile([128, NQT], FP32)
        var_all = consts.tile([128, NQT], FP32)
        std_all = consts.tile([128, NQT], FP32)
        rstd_all = consts.tile([128, NQT], FP32)
        self._zero_bias = zero_bias

        nc.vector.memset(zero_bias, 0.0)
        nc.vector.memset(eps_bias, LN_EPS)
        nc.vector.memset(acc_sb, 0.0)
        make_identity(nc, ident)

        # ---- shared PSUM pools (whole kernel): 3*2 + 1 + 1 = 8 banks ----
        s_ps = ctx.enter_context(tc.tile_pool(name="sps", bufs=3,
                                              space="PSUM"))
        pv_ps = ctx.enter_context(tc.tile_pool(name="pvps", bufs=1,
                                               space="PSUM"))
        wo_ps = ctx.enter_context(tc.tile_pool(name="wops", bufs=1,
                                               space="PSUM"))

        # ---- weight staging (fp32) -> rounded copies ----
        # enc DMAs first: they head the critical chain (transpose -> proj ->
        # scores); weights and masks fill the DMA stream behind them.
        with tc.tile_pool(name="wstage", bufs=1) as wstage:
            enc_nat = wstage.tile([128, NKT * D], FP32)
            enc_nat_r = enc_nat.rearrange("p (t d) -> p t d", t=NKT)
            enc_b_r = enc_b.rearrange("(t p) d -> p t d", p=128)
            for c in range(4):
                nc.sync.dma_start(enc_nat_r[:, c * 4:(c + 1) * 4, :],
                                  enc_b_r[:, c * 4:(c + 1) * 4, :])
            nc.sync.dma_start(
                encq_sb.rearrange("p (t d) -> p t d", t=NQT),
                enc_q.rearrange("(t p) d -> p t d", p=128),
            )
            wq_sb = wstage.tile([D, H * D], FP32)
            wk_sb = wstage.tile([D, H * D], FP32)
            wv_sb = wstage.tile([D, H * D], FP32)
            wo_sb = wstage.tile([D, H * D], FP32)
            nc.sync.dma_start(wk_sb, wk_d[:, :])
            nc.sync.dma_start(wv_sb, wv_d[:, :])
            nc.sync.dma_start(wq_sb, wq_d[:, :])
            # wo in [d, (h m)] layout: block h = Wo_h = wo[h*D:(h+1)*D, :]
            nc.sync.dma_start(
                wo_sb.rearrange("p (h m) -> p h m", h=H),
                wo_d.rearrange("(h d) m -> d h m", h=H),
            )
            nc.vector.tensor_copy(wk_r, wk_sb)
            nc.vector.tensor_copy(wv_r, wv_sb)
            nc.vector.tensor_copy(wq_r, wq_sb)
            nc.vector.tensor_copy(wo16, wo_sb)
            nc.sync.dma_start(lns_sb, bass.AP(lns_d, 0, [[0, 128], [1, D]]))
            nc.sync.dma_start(lnb_sb, bass.AP(lnb_d, 0, [[0, 128], [1, D]]))
            # mask: int32 {0,1} DRAM -> fp16 SBUF via casting gpsimd DGE
            for qt in range(NQT):
                nc.gpsimd.dma_start(
                    mask_all[:, qt * S:(qt + 1) * S],
                    mask_p[qt * 128:(qt + 1) * 128, :])

            # enc transposes (psum from the shared s pool)
            for c in range(2):
                ps = s_ps.tile([128, 1024], FP32, tag="s")
                for j in range(8):
                    t = c * 8 + j
                    nc.tensor.transpose(
                        ps[:, j * 128:(j + 1) * 128],
                        enc_nat[:, t * D:(t + 1) * D], ident)
                nc.scalar.copy(encT_sb[:, c * 1024:(c + 1) * 1024], ps)
            ps = s_ps.tile([128, 1024], FP32, tag="s")
            for t in range(NQT):
                nc.tensor.transpose(
                    ps[:, t * 128:(t + 1) * 128],
                    encq_sb[:, t * D:(t + 1) * D], ident)
            nc.scalar.copy(encqT_sb, ps)

        # ---- working pools ----
        p_pool = ctx.enter_context(tc.tile_pool(name="p", bufs=4))
        af_pool = ctx.enter_context(tc.tile_pool(name="af", bufs=3))
        attnT_pool = ctx.enter_context(tc.tile_pool(name="attnT", bufs=2))
        outw_pool = ctx.enter_context(tc.tile_pool(name="outw", bufs=2))
        stat_pool = ctx.enter_context(tc.tile_pool(name="stat", bufs=8))
        head_pool = ctx.enter_context(tc.tile_pool(name="head", bufs=2))
        junk_pool = ctx.enter_context(tc.tile_pool(name="junk", bufs=2))

        for h in range(H):
            # -- projections for head h (overlap with head h-1's attention) --
            kt_h = head_pool.tile([D, S], F32R, tag="kt")      # [d, k]
            v_h = head_pool.tile([128, NKT * D], FP16, tag="v")  # [k,(kt d)]
            qt_h = head_pool.tile([D, QS], F32R, tag="qt")     # [d, q]
            for c in range(2):
                ps = s_ps.tile([128, 1024], FP32, tag="s")
                for j in range(2):
                    cc = c * 2 + j
                    nc.tensor.matmul(
                        ps[:, j * 512:(j + 1) * 512],
                        wk_r[:, h * D:(h + 1) * D],
                        encT_sb[:, cc * 512:(cc + 1) * 512],
                        start=True, stop=True)
                nc.scalar.copy(kt_h[:, c * 1024:(c + 1) * 1024], ps)
            for c in range(2):
                ps = s_ps.tile([128, 1024], FP32, tag="s")
                for j in range(8):
                    kt = c * 8 + j
                    nc.tensor.matmul(
                        ps[:, j * 128:(j + 1) * 128],
                        encT_sb[:, kt * D:(kt + 1) * D],
                        wv_r[:, h * D:(h + 1) * D],
                        start=True, stop=True)
                nc.scalar.copy(v_h[:, c * 1024:(c + 1) * 1024], ps)
            ps = s_ps.tile([128, 1024], FP32, tag="s")
            for j in range(2):
                nc.tensor.matmul(
                    ps[:, j * 512:(j + 1) * 512],
                    wq_r[:, h * D:(h + 1) * D],
                    encqT_sb[:, j * 512:(j + 1) * 512],
                    start=True, stop=True)
            nc.scalar.copy(qt_h, ps)

            for half in range(2):
                attnT_sb = attnT_pool.tile([128, NKT * 512], FP16,
                                           tag="attnT")
                attnT_r = attnT_sb.rearrange("p (kt q) -> p kt q", kt=NKT)
                for qtl in range(4):
                    qt = half * 4 + qtl
                    self.attention_tile(
                        s_ps, p_pool, af_pool, stat_pool,
                        qt_h, kt_h, mask_all[:, qt * S:(qt + 1) * S],
                        attnT_r, attn_o, h, qt, qtl)
                # -- PV: out_h^T[d, q512] accumulated over kt --
                pv_t = pv_ps.tile([128, 512], FP32, tag="pv")
                for kt in range(NKT):
                    nc.tensor.matmul(
                        pv_t,
                        v_h[:, kt * D:(kt + 1) * D],
                        attnT_sb[:, kt * 512:(kt + 1) * 512],
                        start=(kt == 0), stop=(kt == NKT - 1))
                outw_t = outw_pool.tile([128, 512], FP16, tag="outw")
                nc.scalar.copy(outw_t, pv_t)
                # -- Wo: out_pre[q, dm] for 4 q-tiles --
                wo_t = wo_ps.tile([128, 512], FP32, tag="wo")
                for i in range(4):
                    nc.tensor.matmul(
                        wo_t[:, i * 128:(i + 1) * 128],
                        outw_t[:, i * 128:(i + 1) * 128],
                        wo16[:, h * D:(h + 1) * D],
                        start=True, stop=True)
                dst = acc_sb[:, half * 512:(half + 1) * 512]
                nc.vector.tensor_add(dst, dst, wo_t)

                # -- residual + LayerNorm for this half once all heads done --
                if h == H - 1:
                    self.layer_norm_half(
                        half, acc_sb, encq_sb, lns_sb, lnb_sb, eps_bias,
                        mu_all, var_all, std_all, rstd_all, stat_pool,
                        junk_pool, out_o)

    def layer_norm_half(self, half, acc_sb, encq_sb, lns_sb, lnb_sb, eps_bias,
                        mu_all, var_all, std_all, rstd_all, stat_pool,
                        junk_pool, out_o):
        nc = self.nc
        q0t = half * 4
        for qt in range(q0t, q0t + 4):
            x = acc_sb[:, qt * D:(qt + 1) * D]
            sum_t = stat_pool.tile([128, 1], FP32, tag="lnsum")
            nc.vector.scalar_tensor_tensor(
                out=x, in0=x, scalar=0.0, in1=encq_sb[:, qt * D:(qt + 1) * D],
                op0=OP.add, op1=OP.add, accum_out=sum_t)
            nc.vector.tensor_scalar_mul(mu_all[:, qt:qt + 1], sum_t, 1.0 / D)
            nc.vector.tensor_scalar(
                out=x, in0=x, scalar1=mu_all[:, qt:qt + 1], scalar2=None,
                op0=OP.subtract)
            sq_t = junk_pool.tile([128, D], FP32, tag="junk")
            nc.vector.scalar_tensor_tensor(
                out=sq_t, in0=x, scalar=1.0, in1=x,
                op0=OP.mult, op1=OP.mult, accum_out=var_all[:, qt:qt + 1])
        # var_all holds sum(xc^2); sqrt(sum/D + eps) via ACT's free affine
        nc.scalar.activation(
            std_all[:, q0t:q0t + 4], var_all[:, q0t:q0t + 4], AF.Sqrt,
            bias=eps_bias, scale=1.0 / D)
        nc.vector.reciprocal(
            rstd_all[:, q0t:q0t + 4], std_all[:, q0t:q0t + 4])
        for qt in range(q0t, q0t + 4):
            x = acc_sb[:, qt * D:(qt + 1) * D]
            nc.vector.tensor_scalar(
                out=x, in0=x, scalar1=rstd_all[:, qt:qt + 1], scalar2=None,
                op0=OP.mult)
            nc.vector.tensor_mul(x, x, lns_sb)
            nc.vector.tensor_add(x, x, lnb_sb)
        nc.sync.dma_start(
            out_o.rearrange("(t p) d -> p t d", p=128)[:, q0t:q0t + 4, :],
            acc_sb.rearrange("p (t d) -> p t d", t=NQT)[:, q0t:q0t + 4, :],
        )

    def attention_tile(self, s_ps, p_pool, af_pool, stat_pool,
                       qt_h, kt_h, m_t, attnT_r, attn_o, h, qt, qtl):
        """softmax row block: scores -> exp(fp16) -> mask+rowsum ->
        normalize in place -> f32 DMA out + XBAR transpose into attnT."""
        nc = self.nc
        p_t = p_pool.tile([128, S], FP16, tag="p")
        lhs = qt_h[:, qt * 128:(qt + 1) * 128]
        for c in range(2):
            s_t = s_ps.tile([128, 1024], FP32, tag="s")
            for j in range(2):
                nc.tensor.matmul(
                    s_t[:, j * 512:(j + 1) * 512],
                    lhs,
                    kt_h[:, (c * 2 + j) * 512:(c * 2 + j + 1) * 512],
                    start=True, stop=True)
            nc.scalar.activation(
                p_t[:, c * 1024:(c + 1) * 1024], s_t, AF.Exp,
                bias=self._zero_bias, scale=SCALE)
        rs_t = stat_pool.tile([128, 1], FP32, tag="rs")
        nc.vector.scalar_tensor_tensor(
            out=p_t, in0=p_t, scalar=1.0, in1=m_t,
            op0=OP.mult, op1=OP.mult, accum_out=rs_t)
        rinv_t = stat_pool.tile([128, 1], FP32, tag="rinv")
        nc.vector.reciprocal(rinv_t, rs_t)
        nc.vector.tensor_scalar(
            out=p_t, in0=p_t, scalar1=rinv_t, scalar2=None, op0=OP.mult)
        attn_f = af_pool.tile([128, S], FP32, tag="af")
        nc.vector.tensor_copy(attn_f, p_t)
        nc.sync.dma_start(attn_o[h, qt * 128:(qt + 1) * 128, :], attn_f)
        nc.sync.dma_start_transpose(
            attnT_r[:, :, qtl * 128:(qtl + 1) * 128], p_t)


# ---------------------------------------------------------------------------
_NC_CACHE = None


def _get_nc():
    global _NC_CACHE
    if _NC_CACHE is None:
        _NC_CACHE = build_bass()
    return _NC_CACHE


def make_in_maps(inputs):
    enc = np.asarray(inputs["enc"], np.float32)
    mask = np.asarray(inputs["mask"], np.int32)
    wq = np.asarray(inputs["Wq"], np.float32)
    wk = np.asarray(inputs["Wk"], np.float32)
    wv = np.asarray(inputs["Wv"], np.float32)
    wo = np.asarray(inputs["Wo"], np.float32)
    lns = np.asarray(inputs["ln_scale"], np.float32)
    lnb = np.asarray(inputs["ln_bias"], np.float32)

    in_maps = []
    for c in range(8):
        b, half = divmod(c, 2)
        q0 = half * QS
        in_maps.append({
            "enc_b": np.ascontiguousarray(enc[b]),
            "enc_q": np.ascontiguousarray(enc[b, q0:q0 + QS]),
            "mask_p": np.ascontiguousarray(mask[b, 0, q0:q0 + QS, :]),
            "wq": wq, "wk": wk, "wv": wv, "wo": wo,
            "ln_scale": lns, "ln_bias": lnb,
        })
    return in_maps


def kernel(**inputs):
    nc = _get_nc()
    in_maps = make_in_maps(inputs)
    res = run_bass_kernel_spmd(nc, in_maps, core_ids=list(range(8)))
    attn = np.empty((B, H, S, S), np.float32)
    out = np.empty((B, S, D), np.float32)
    for c in range(8):
        b, half = divmod(c, 2)
        q0 = half * QS
        attn[b, :, q0:q0 + QS, :] = res.results[c]["attn_o"]
        out[b, q0:q0 + QS, :] = res.results[c]["out_o"]
    return out, attn


# revision 38
# speedup vs baseline: 11.6049x; 11.6049x over previous
"""Multi-head attention layer (B=4, S=2048, H=8, D=128) on 8 trn2 NeuronCores.

Sharding: core c handles batch b = c//2 and query half c%2 (1024 query rows).
Each core computes K/V over all 2048 keys of its batch, its 1024x2048 slice of
the attention matrix for all 8 heads, and its slice of the final
residual+LayerNorm output.  Outputs are assembled on the host by pure
concatenation - no cross-core reduction.

Pipeline: heads stream one at a time; head h+1's Q/K/V projections overlap
head h's softmax/PV work (all PSUM pools are shared, so nothing serializes on
allocation).  The random 0/1 int32 mask is DMA-cast to fp16 once up front by
the gpsimd DGE.  Score chain (projections + Q@K) runs in float32r (full-rate
fp32 with ~1e-4 rounding); the value chain (exp output, attn weights, V, Wo)
runs fp16; the attention-probability output, residual and LayerNorm are fp32.
The big probability transpose for attn@V rides the DMA XBAR transpose engine
instead of the tensor engine.
"""

import math
from contextlib import ExitStack

import numpy as np

import concourse.bass as bass
import concourse.mybir as mybir
import concourse.tile as tile
from concourse import bacc
from concourse.bass_utils import run_bass_kernel_spmd
from concourse.masks import make_identity

B, S, D, H = 4, 2048, 128, 8
QS = S // 2            # 1024 query rows per core
NQT = QS // 128        # 8 query tiles per core
NKT = S // 128         # 16 key tiles
SCALE = 1.0 / math.sqrt(D)
LN_EPS = 1e-6

FP32 = mybir.dt.float32
FP16 = mybir.dt.float16
F32R = mybir.dt.float32r
I32 = mybir.dt.int32
AF = mybir.ActivationFunctionType
OP = mybir.AluOpType


def build_bass():
    nc = bacc.Bacc(trn_type="TRN2")

    enc_b = nc.dram_tensor("enc_b", [S, D], FP32, kind="ExternalInput")
    enc_q = nc.dram_tensor("enc_q", [QS, D], FP32, kind="ExternalInput")
    mask_p = nc.dram_tensor("mask_p", [QS, S], I32, kind="ExternalInput")
    wq_d = nc.dram_tensor("wq", [D, H * D], FP32, kind="ExternalInput")
    wk_d = nc.dram_tensor("wk", [D, H * D], FP32, kind="ExternalInput")
    wv_d = nc.dram_tensor("wv", [D, H * D], FP32, kind="ExternalInput")
    wo_d = nc.dram_tensor("wo", [H * D, D], FP32, kind="ExternalInput")
    lns_d = nc.dram_tensor("ln_scale", [D], FP32, kind="ExternalInput")
    lnb_d = nc.dram_tensor("ln_bias", [D], FP32, kind="ExternalInput")
    attn_o = nc.dram_tensor("attn_o", [H, QS, S], FP32, kind="ExternalOutput")
    out_o = nc.dram_tensor("out_o", [QS, D], FP32, kind="ExternalOutput")

    with TileKernel(nc) as tk:
        tk.run(enc_b, enc_q, mask_p, wq_d, wk_d, wv_d, wo_d, lns_d, lnb_d,
               attn_o, out_o)
    nc.finalize()
    return nc


class TileKernel:
    def __init__(self, nc):
        self.nc = nc
        self.ctx = ExitStack()
        self.tc = None

    def __enter__(self):
        self.tc = self.ctx.enter_context(tile.TileContext(self.nc))
        return self

    def __exit__(self, *exc):
        return self.ctx.__exit__(*exc)

    def run(self, enc_b, enc_q, mask_p, wq_d, wk_d, wv_d, wo_d, lns_d, lnb_d,
            attn_o, out_o):
        nc, tc, ctx = self.nc, self.tc, self.ctx

        consts = ctx.enter_context(tc.tile_pool(name="consts", bufs=1))

        # ---- persistent tiles ----
        wq_r = consts.tile([D, H * D], F32R)
        wk_r = consts.tile([D, H * D], F32R)
        wv_r = consts.tile([D, H * D], F32R)
        wo16 = consts.tile([D, H * D], FP16)
        lns_sb = consts.tile([128, D], FP32)
        lnb_sb = consts.tile([128, D], FP32)
        ident = consts.tile([128, 128], FP32)
        encq_sb = consts.tile([128, NQT * D], FP32)
        encT_sb = consts.tile([D, S], F32R)     # enc_b transposed [d, seq]
        encqT_sb = consts.tile([D, QS], F32R)   # enc_q transposed [d, q]
        acc_sb = consts.tile([128, NQT * D], FP32)  # sum_h attn@V@Wo, [q, d]
        mask_all = consts.tile([128, NQT * S], FP16)  # all 8 q-tiles of mask
        zero_bias = consts.tile([128, 1], FP32)
        eps_bias = consts.tile([128, 1], FP32)
        mu_all = consts.tile([128, NQT], FP32)
        var_all = consts.tile([128, NQT], FP32)
        std_all = consts.tile([128, NQT], FP32)
        rstd_all = consts.tile([128, NQT], FP32)
        self._zero_bias = zero_bias

        nc.vector.memset(zero_bias, 0.0)
        nc.vector.memset(eps_bias, LN_EPS)
        nc.vector.memset(acc_sb, 0.0)
        make_identity(nc, ident)
        # warm the ACT exp table set during the input DMAs
        warm_t = consts.tile([128, 1], FP32)
        nc.scalar.activation(warm_t, zero_bias, AF.Exp, bias=zero_bias)

        # ---- shared PSUM pools (whole kernel): 3*2 + 1 + 1 = 8 banks ----
        s_ps = ctx.enter_context(tc.tile_pool(name="sps", bufs=3,
                                              space="PSUM"))
        pv_ps = ctx.enter_context(tc.tile_pool(name="pvps", bufs=1,
                                               space="PSUM"))
        wo_ps = ctx.enter_context(tc.tile_pool(name="wops", bufs=1,
                                               space="PSUM"))

        # ---- weight staging (fp32) -> rounded copies ----
        # enc DMAs first: they head the critical chain (transpose -> proj ->
        # scores); weights and masks fill the DMA stream behind them.
        with tc.tile_pool(name="wstage", bufs=1) as wstage:
            enc_nat = wstage.tile([128, NKT * D], FP32)
            enc_nat_r = enc_nat.rearrange("p (t d) -> p t d", t=NKT)
            enc_b_r = enc_b.rearrange("(t p) d -> p t d", p=128)
            for c in range(4):
                nc.sync.dma_start(enc_nat_r[:, c * 4:(c + 1) * 4, :],
                                  enc_b_r[:, c * 4:(c + 1) * 4, :])
            nc.sync.dma_start(
                encq_sb.rearrange("p (t d) -> p t d", t=NQT),
                enc_q.rearrange("(t p) d -> p t d", p=128),
            )
            wq_sb = wstage.tile([D, H * D], FP32)
            wk_sb = wstage.tile([D, H * D], FP32)
            wv_sb = wstage.tile([D, H * D], FP32)
            wo_sb = wstage.tile([D, H * D], FP32)
            nc.sync.dma_start(wk_sb, wk_d[:, :])
            nc.sync.dma_start(wq_sb, wq_d[:, :])
            # first-half masks next: they gate the first softmax tiles
            # (int32 {0,1} DRAM -> fp16 SBUF via casting gpsimd DGE)
            for qt in range(4):
                nc.gpsimd.dma_start(
                    mask_all[:, qt * S:(qt + 1) * S],
                    mask_p[qt * 128:(qt + 1) * 128, :])
            nc.sync.dma_start(wv_sb, wv_d[:, :])
            # wo in [d, (h m)] layout: block h = Wo_h = wo[h*D:(h+1)*D, :]
            nc.sync.dma_start(
                wo_sb.rearrange("p (h m) -> p h m", h=H),
                wo_d.rearrange("(h d) m -> d h m", h=H),
            )
            nc.vector.tensor_copy(wk_r, wk_sb)
            nc.vector.tensor_copy(wq_r, wq_sb)
            nc.vector.tensor_copy(wv_r, wv_sb)
            nc.vector.tensor_copy(wo16, wo_sb)
            nc.sync.dma_start(lns_sb, bass.AP(lns_d, 0, [[0, 128], [1, D]]))
            nc.sync.dma_start(lnb_sb, bass.AP(lnb_d, 0, [[0, 128], [1, D]]))
            for qt in range(4, NQT):
                nc.gpsimd.dma_start(
                    mask_all[:, qt * S:(qt + 1) * S],
                    mask_p[qt * 128:(qt + 1) * 128, :])

            # enc transposes (psum from the shared s pool)
            for c in range(2):
                ps = s_ps.tile([128, 1024], FP32, tag="s")
                for j in range(8):
                    t = c * 8 + j
                    nc.tensor.transpose(
                        ps[:, j * 128:(j + 1) * 128],
                        enc_nat[:, t * D:(t + 1) * D], ident)
                nc.scalar.copy(encT_sb[:, c * 1024:(c + 1) * 1024], ps)
            ps = s_ps.tile([128, 1024], FP32, tag="s")
            for t in range(NQT):
                nc.tensor.transpose(
                    ps[:, t * 128:(t + 1) * 128],
                    encq_sb[:, t * D:(t + 1) * D], ident)
            nc.scalar.copy(encqT_sb, ps)

        # ---- working pools ----
        p_pool = ctx.enter_context(tc.tile_pool(name="p", bufs=6))
        af_pool = ctx.enter_context(tc.tile_pool(name="af", bufs=4))
        attnT_pool = ctx.enter_context(tc.tile_pool(name="attnT", bufs=2))
        outw_pool = ctx.enter_context(tc.tile_pool(name="outw", bufs=2))
        stat_pool = ctx.enter_context(tc.tile_pool(name="stat", bufs=8))
        head_pool = ctx.enter_context(tc.tile_pool(name="head", bufs=2))
        junk_pool = ctx.enter_context(tc.tile_pool(name="junk", bufs=2))

        for h in range(H):
            # -- projections for head h (overlap with head h-1's attention) --
            kt_h = head_pool.tile([D, S], F32R, tag="kt")      # [d, k]
            v_h = head_pool.tile([128, NKT * D], FP16, tag="v")  # [k,(kt d)]
            qt_h = head_pool.tile([D, QS], F32R, tag="qt")     # [d, q]
            for c in range(2):
                ps = s_ps.tile([128, 1024], FP32, tag="s")
                for j in range(2):
                    cc = c * 2 + j
                    nc.tensor.matmul(
                        ps[:, j * 512:(j + 1) * 512],
                        wk_r[:, h * D:(h + 1) * D],
                        encT_sb[:, cc * 512:(cc + 1) * 512],
                        start=True, stop=True)
                nc.scalar.copy(kt_h[:, c * 1024:(c + 1) * 1024], ps)
            for c in range(2):
                ps = s_ps.tile([128, 1024], FP32, tag="s")
                for j in range(8):
                    kt = c * 8 + j
                    nc.tensor.matmul(
                        ps[:, j * 128:(j + 1) * 128],
                        encT_sb[:, kt * D:(kt + 1) * D],
                        wv_r[:, h * D:(h + 1) * D],
                        start=True, stop=True)
                nc.scalar.copy(v_h[:, c * 1024:(c + 1) * 1024], ps)
            ps = s_ps.tile([128, 1024], FP32, tag="s")
            for j in range(2):
                nc.tensor.matmul(
                    ps[:, j * 512:(j + 1) * 512],
                    wq_r[:, h * D:(h + 1) * D],
                    encqT_sb[:, j * 512:(j + 1) * 512],
                    start=True, stop=True)
            nc.scalar.copy(qt_h, ps)

            for half in range(2):
                attnT_sb = attnT_pool.tile([128, NKT * 512], FP16,
                                           tag="attnT")
                attnT_r = attnT_sb.rearrange("p (kt q) -> p kt q", kt=NKT)
                for qtl in range(4):
                    qt = half * 4 + qtl
                    self.attention_tile(
                        s_ps, p_pool, af_pool, stat_pool,
                        qt_h, kt_h, mask_all[:, qt * S:(qt + 1) * S],
                        attnT_r, attn_o, h, qt, qtl)
                # -- PV: out_h^T[d, q512] accumulated over kt --
                pv_t = pv_ps.tile([128, 512], FP32, tag="pv")
                for kt in range(NKT):
                    nc.tensor.matmul(
                        pv_t,
                        v_h[:, kt * D:(kt + 1) * D],
                        attnT_sb[:, kt * 512:(kt + 1) * 512],
                        start=(kt == 0), stop=(kt == NKT - 1))
                outw_t = outw_pool.tile([128, 512], FP16, tag="outw")
                nc.scalar.copy(outw_t, pv_t)
                # -- Wo: out_pre[q, dm] for 4 q-tiles --
                wo_t = wo_ps.tile([128, 512], FP32, tag="wo")
                for i in range(4):
                    nc.tensor.matmul(
                        wo_t[:, i * 128:(i + 1) * 128],
                        outw_t[:, i * 128:(i + 1) * 128],
                        wo16[:, h * D:(h + 1) * D],
                        start=True, stop=True)
                dst = acc_sb[:, half * 512:(half + 1) * 512]
                nc.vector.tensor_add(dst, dst, wo_t)

        # -- residual + LayerNorm (single block: one Sqrt table switch) --
        for half in range(2):
            self.layer_norm_half(
                half, acc_sb, encq_sb, lns_sb, lnb_sb, eps_bias,
                mu_all, var_all, std_all, rstd_all, stat_pool,
                junk_pool, out_o)

    def layer_norm_half(self, half, acc_sb, encq_sb, lns_sb, lnb_sb, eps_bias,
                        mu_all, var_all, std_all, rstd_all, stat_pool,
                        junk_pool, out_o):
        nc = self.nc
        q0t = half * 4
        for qt in range(q0t, q0t + 4):
            x = acc_sb[:, qt * D:(qt + 1) * D]
            sum_t = stat_pool.tile([128, 1], FP32, tag="lnsum")
            nc.vector.scalar_tensor_tensor(
                out=x, in0=x, scalar=0.0, in1=encq_sb[:, qt * D:(qt + 1) * D],
                op0=OP.add, op1=OP.add, accum_out=sum_t)
            nc.vector.tensor_scalar_mul(mu_all[:, qt:qt + 1], sum_t, 1.0 / D)
            nc.vector.tensor_scalar(
                out=x, in0=x, scalar1=mu_all[:, qt:qt + 1], scalar2=None,
                op0=OP.subtract)
            sq_t = junk_pool.tile([128, D], FP32, tag="junk")
            nc.vector.scalar_tensor_tensor(
                out=sq_t, in0=x, scalar=1.0, in1=x,
                op0=OP.mult, op1=OP.mult, accum_out=var_all[:, qt:qt + 1])
        # var_all holds sum(xc^2); sqrt(sum/D + eps) via ACT's free affine
        nc.scalar.activation(
            std_all[:, q0t:q0t + 4], var_all[:, q0t:q0t + 4], AF.Sqrt,
            bias=eps_bias, scale=1.0 / D)
        nc.vector.reciprocal(
            rstd_all[:, q0t:q0t + 4], std_all[:, q0t:q0t + 4])
        for qt in range(q0t, q0t + 4):
            x = acc_sb[:, qt * D:(qt + 1) * D]
            # y = (xc * rstd) * ln_scale, then + ln_bias
            nc.vector.scalar_tensor_tensor(
                out=x, in0=x, scalar=rstd_all[:, qt:qt + 1], in1=lns_sb,
                op0=OP.mult, op1=OP.mult)
            nc.vector.tensor_add(x, x, lnb_sb)
        nc.sync.dma_start(
            out_o.rearrange("(t p) d -> p t d", p=128)[:, q0t:q0t + 4, :],
            acc_sb.rearrange("p (t d) -> p t d", t=NQT)[:, q0t:q0t + 4, :],
        )

    def attention_tile(self, s_ps, p_pool, af_pool, stat_pool,
                       qt_h, kt_h, m_t, attnT_r, attn_o, h, qt, qtl):
        """softmax row block: scores -> exp(fp16) -> mask+rowsum ->
        normalize in place -> f32 DMA out + XBAR transpose into attnT."""
        nc = self.nc
        p_t = p_pool.tile([128, S], FP16, tag="p")
        lhs = qt_h[:, qt * 128:(qt + 1) * 128]
        for c in range(2):
            s_t = s_ps.tile([128, 1024], FP32, tag="s")
            for j in range(2):
                nc.tensor.matmul(
                    s_t[:, j * 512:(j + 1) * 512],
                    lhs,
                    kt_h[:, (c * 2 + j) * 512:(c * 2 + j + 1) * 512],
                    start=True, stop=True)
            nc.scalar.activation(
                p_t[:, c * 1024:(c + 1) * 1024], s_t, AF.Exp,
                bias=self._zero_bias, scale=SCALE)
        rs_t = stat_pool.tile([128, 1], FP32, tag="rs")
        nc.vector.scalar_tensor_tensor(
            out=p_t, in0=p_t, scalar=1.0, in1=m_t,
            op0=OP.mult, op1=OP.mult, accum_out=rs_t)
        rinv_t = stat_pool.tile([128, 1], FP32, tag="rinv")
        nc.vector.reciprocal(rinv_t, rs_t)
        nc.vector.tensor_scalar(
            out=p_t, in0=p_t, scalar1=rinv_t, scalar2=None, op0=OP.mult)
        attn_f = af_pool.tile([128, S], FP32, tag="af")
        nc.vector.tensor_copy(attn_f, p_t)
        nc.sync.dma_start(attn_o[h, qt * 128:(qt + 1) * 128, :], attn_f)
        nc.sync.dma_start_transpose(
            attnT_r[:, :, qtl * 128:(qtl + 1) * 128], p_t)


# ---------------------------------------------------------------------------
_NC_CACHE = None


def _get_nc():
    global _NC_CACHE
    if _NC_CACHE is None:
        _NC_CACHE = build_bass()
    return _NC_CACHE


def make_in_maps(inputs):
    enc = np.asarray(inputs["enc"], np.float32)
    mask = np.asarray(inputs["mask"], np.int32)
    wq = np.asarray(inputs["Wq"], np.float32)
    wk = np.asarray(inputs["Wk"], np.float32)
    wv = np.asarray(inputs["Wv"], np.float32)
    wo = np.asarray(inputs["Wo"], np.float32)
    lns = np.asarray(inputs["ln_scale"], np.float32)
    lnb = np.asarray(inputs["ln_bias"], np.float32)

    in_maps = []
    for c in range(8):
        b, half = divmod(c, 2)
        q0 = half * QS
        in_maps.append({
            "enc_b": np.ascontiguousarray(enc[b]),
            "enc_q": np.ascontiguousarray(enc[b, q0:q0 + QS]),
            "mask_p": np.ascontiguousarray(mask[b, 0, q0:q0 + QS, :]),
            "wq": wq, "wk": wk, "wv": wv, "wo": wo,
            "ln_scale": lns, "ln_bias": lnb,
        })
    return in_maps


def kernel(**inputs):
    nc = _get_nc()
    in_maps = make_in_maps(inputs)
    res = run_bass_kernel_spmd(nc, in_maps, core_ids=list(range(8)))
    attn = np.empty((B, H, S, S), np.float32)
    out = np.empty((B, S, D), np.float32)
    for c in range(8):
        b, half = divmod(c, 2)
        q0 = half * QS
        attn[b, :, q0:q0 + QS, :] = res.results[c]["attn_o"]
        out[b, q0:q0 + QS, :] = res.results[c]["out_o"]
    return out, attn
